# revision 6
# baseline (speedup 1.0000x reference)
"""nn_CRFLayer: CRF Viterbi decode on 8 Trainium2 NeuronCores — packed version.

Key idea: sent_lengths are uniform 1..512 (mean ~256), so half of all (b,t)
positions are padding. The host bin-packs the 512 sequences into 256 "bins"
(lane-slots) of T_PACK steps (multiple sequences concatenated in time, with
alpha resets at the boundaries); each core runs 32 bins. Partition layout:
p = 4*bin_local + q, q = c-quarter (4 copies of each bin), so each lane does
12 cur x 48 prev per step instead of 24 x 48 — half the element work of the
unpacked kernel, with no padded steps.

All heavy ops run on the DVE (the Pool engine's software TTs contend with
DVE for SBUF and slow both ~2x when overlapped; measured). Per step:
  add (scores = trans_q + alpha_bcast), segmented reduce_max, one fused
  scalar_tensor_tensor alpha update (maxv * not_reset + emit — handles both
  the recurrence and the boundary reset), two stream_shuffles (4-way
  alpha all-gather). Backpointer extraction (is_ge mask -> bf16 mult by
  encoded iota -> segmented reduce_min, exact first-index argmax) is
  batched x4 steps and lags the alpha chain.

Sequence boundaries in the backward chase are handled with constant-map
backpointers: a batched post-pass computes AT[bin, t] = argmax_c alpha[t]
for every t, and the backward merges AT[t-1] into the bp stream at boundary
steps, so the chase "snaps" to each sequence's last tag with a plain 2-op
per-step chain (one-hot dot-accumulate + is_equal regeneration).

All value-producing float ops are the same single fp32 adds as the
reference, so decoded tags match bitwise-exactly.
"""

import numpy as np
import ml_dtypes

import concourse.bass as bass
import concourse.mybir as mybir
from concourse.tile import TileContext

AL = mybir.AluOpType
F32 = mybir.dt.float32
BF16 = mybir.dt.bfloat16
I32 = mybir.dt.int32

D = 48
Q = 12              # cur per lane
NBIN = 32           # bins per core
N_CORES = 8
B = 512
T_IN = 512
CK = 32             # forward DMA chunk (steps)
BATCH = 16          # bp-extraction batch (steps)
CKB = 32            # backward chunk (steps)

ROT1_MASK = [(i & ~3) | ((i + 1) & 3) for i in range(32)]
ROT2_MASK = [(i & ~3) | ((i + 2) & 3) for i in range(32)]


# ---------------------------------------------------------------------------
# host-side packing
# ---------------------------------------------------------------------------

def pack_bins(lengths: np.ndarray) -> tuple[int, list[list[tuple[int, int, int]]]]:
    """FFD-pack sequences into NBIN*N_CORES bins. Returns (T_PACK, bins) where
    bins[i] = [(orig_idx, start, length), ...]."""
    nbins = NBIN * N_CORES
    order = np.argsort(-lengths, kind="stable")
    t_pack = max(512, int(np.ceil(lengths.sum() / nbins / 16) * 16))
    while True:
        fills = [0] * nbins
        bins = [[] for _ in range(nbins)]
        ok = True
        for idx in order:
            L = int(lengths[idx])
            for i in range(nbins):
                if fills[i] + L <= t_pack:
                    bins[i].append((int(idx), fills[i], L))
                    fills[i] += L
                    break
            else:
                ok = False
                break
        if ok:
            return t_pack, bins
        t_pack += 16


def make_consts(trans: np.ndarray, t_pack: int) -> dict[str, np.ndarray]:
    trans = np.asarray(trans, dtype=np.float32)
    trans_q = np.zeros((128, Q, D), dtype=np.float32)
    iota_q = np.zeros((128, Q, D), dtype=np.float32)
    for q in range(4):
        prev = (Q * q + np.arange(D)) % D          # jslot -> real j
        cur = Q * q + np.arange(Q)
        block = trans[prev][:, cur].T              # [Q, D]
        enc = np.broadcast_to((prev - 64.0)[None, :], (Q, D))
        for bl in range(NBIN):
            p = 4 * bl + q
            trans_q[p] = block
            iota_q[p] = enc
    at_enc = np.broadcast_to(
        (np.arange(D, dtype=np.float32) - 64.0)[None, :], (128, D)
    ).copy()                                        # (q,c) -> 12q+c - 64
    iota_bw = np.broadcast_to(
        (np.arange(D, dtype=np.float32) - 64.0)[None, :], (NBIN, D)
    ).copy()
    return {
        "trans_q": trans_q.reshape(128, Q * D),
        "iota_q": iota_q.reshape(128, Q * D).astype(ml_dtypes.bfloat16),
        "at_enc": at_enc,
        "iota_bw": iota_bw,
    }


def make_core_inputs(logits, bins_core, consts, t_pack) -> dict[str, np.ndarray]:
    """bins_core: 32 bins for this core, each [(orig_idx, start, L), ...]."""
    emit = np.zeros((128, t_pack, Q), dtype=np.float32)
    nr = np.ones((NBIN, t_pack), dtype=np.float32)
    bm = np.zeros((NBIN, t_pack), dtype=np.float32)
    for bl, seqs in enumerate(bins_core):
        fill = 0
        for (idx, s, L) in seqs:
            for q in range(4):
                emit[4 * bl + q, s:s + L, :] = logits[idx, :L, Q * q:Q * q + Q]
            nr[bl, s] = 0.0
            if s >= 1:
                bm[bl, s] = 1.0
            fill = s + L
        if fill < t_pack:          # junk start is a boundary too
            nr[bl, fill] = 0.0
            if fill >= 1:
                bm[bl, fill] = 1.0
    nr_il = np.repeat(nr, 4, axis=0)               # [128, T]
    nbm = 1.0 - bm
    return dict(
        consts,
        emit=np.ascontiguousarray(emit),
        nr_il=np.ascontiguousarray(nr_il),
        bm=np.ascontiguousarray(bm),
        nbm=np.ascontiguousarray(nbm),
    )


# ---------------------------------------------------------------------------
# kernel
# ---------------------------------------------------------------------------

def crf_kernel(tc: TileContext, outs, ins, T: int):
    nc = tc.nc
    TC = T // 4                     # per-(bl,tc) chunk for the AT pass

    emit_d = ins["emit"]            # [128, T, Q] dram f32
    tags_out = outs["tags"]         # [NBIN, T] dram i32

    bp_dram = nc.dram_tensor("bp_scratch", [128, T, Q], BF16, kind="Internal").ap()
    ah_dram = nc.dram_tensor("ah_scratch", [128, T, Q], F32, kind="Internal").ap()
    at_dram = nc.dram_tensor("at_scratch", [NBIN, T + 1], F32, kind="Internal").ap()

    with (
        tc.tile_pool(name="persist", bufs=1) as pp,
        tc.tile_pool(name="chunks", bufs=3) as cp,
        tc.tile_pool(name="work", bufs=1) as wp,
    ):
        # ---- persistent constants ----
        trans_q = pp.tile([128, Q, D], F32, tag="trans_q")
        nc.sync.dma_start(trans_q[:].rearrange("p a b -> p (a b)"), ins["trans_q"])
        iota_q = pp.tile([128, Q, D], BF16, tag="iota_q")
        nc.sync.dma_start(iota_q[:].rearrange("p a b -> p (a b)"), ins["iota_q"])
        at_enc = pp.tile([128, D], F32, tag="at_enc")
        nc.sync.dma_start(at_enc[:], ins["at_enc"])
        iota_bw = pp.tile([NBIN, D], F32, tag="iota_bw")
        nc.sync.dma_start(iota_bw[:], ins["iota_bw"])
        nr_il = pp.tile([128, T], F32, tag="nr_il")
        nc.sync.dma_start(nr_il[:], ins["nr_il"])
        bm = pp.tile([NBIN, T], F32, tag="bm")
        nc.sync.dma_start(bm[:], ins["bm"])
        nbm = pp.tile([NBIN, T], F32, tag="nbm")
        nc.sync.dma_start(nbm[:], ins["nbm"])

        aring = pp.tile([128, 8, D], F32, tag="aring")
        nc.vector.memset(aring[:], 0.0)

        srings = [pp.tile([128, BATCH, Q, D], F32, name=f"sring{i}", tag=f"sring{i}") for i in range(2)]
        mrings = [pp.tile([128, BATCH, Q], F32, name=f"mring{i}", tag=f"mring{i}") for i in range(2)]

        # ---- forward scan ----
        ah_v4 = ah_dram.rearrange("(b q) t c -> b q t c", q=4)
        atpool_ctx = tc.tile_pool(name="atpool", bufs=1)
        ap = atpool_ctx.__enter__()
        atile = ap.tile([128, TC, 4, Q], F32, tag="atile")
        emit_tiles = {}

        def fetch_emit(t0):
            et = cp.tile([128, CK, Q], F32, name=f"emit_{t0}", tag="emit_ch")
            nc.sync.dma_start(et[:], emit_d[:, t0:t0 + CK, :])
            emit_tiles[t0] = et

        fetch_emit(0)
        for t0 in range(0, T, CK):
            if t0 + CK < T:
                fetch_emit(t0 + CK)
            emit_ch = emit_tiles.pop(t0)
            bp_ch = cp.tile([128, CK, Q], BF16, tag="bp_ch")
            for t in range(t0, t0 + CK):
                r = (t // BATCH) % 2
                k = t % BATCH
                kc = t - t0
                a_prev = aring[:, (t + 7) % 8, :]
                a_next = aring[:, t % 8, :]
                sc = srings[r]
                mv = mrings[r]
                a_b = a_prev.unsqueeze(1).broadcast_to([128, Q, D])
                nc.vector.tensor_tensor(out=sc[:, k], in0=trans_q[:], in1=a_b, op=AL.add)
                nc.vector.tensor_reduce(
                    out=mv[:, k], in_=sc[:, k], axis=mybir.AxisListType.X, op=AL.max,
                )
                # alpha = maxv * not_reset + emit  (reset -> alpha = emit)
                nc.vector.scalar_tensor_tensor(
                    out=a_next[0:128, 0:Q], in0=mv[:, k], scalar=nr_il[:, t:t + 1],
                    in1=emit_ch[:, kc, :], op0=AL.mult, op1=AL.add,
                )
                nc.vector.stream_shuffle(a_next[0:128, Q:2 * Q], a_next[0:128, 0:Q], mask=ROT1_MASK)
                nc.vector.stream_shuffle(a_next[0:128, 2 * Q:4 * Q], a_next[0:128, 0:2 * Q], mask=ROT2_MASK)
                if t % 4 == 3:
                    s4 = (t - 3) % 8
                    nc.sync.dma_start(
                        ah_dram[:, t - 3:t + 1, :], aring[:, s4:s4 + 4, 0:Q]
                    )
                if k == BATCH - 1:
                    # batched bp extraction for steps t-BATCH+1..t
                    mask4 = wp.tile([128, BATCH, Q, D], BF16, tag="mask4")
                    sc_v = sc[:].rearrange("p b c j -> p (b c) j")
                    mv_v = mv[:].rearrange("p b c -> p (b c)").unsqueeze(2)
                    nc.vector.tensor_tensor(
                        out=mask4[:].rearrange("p b c j -> p (b c) j"), in0=sc_v,
                        in1=mv_v.broadcast_to([128, BATCH * Q, D]), op=AL.is_ge,
                    )
                    nc.vector.tensor_tensor(
                        out=mask4[:], in0=mask4[:],
                        in1=iota_q[:].unsqueeze(1).broadcast_to([128, BATCH, Q, D]),
                        op=AL.mult,
                    )
                    # segmented min via bf16 TT tree (2x DVE mode; TR has no 2x)
                    fv = mask4[:].rearrange("p b c j -> p (b c) j")
                    nc.vector.tensor_tensor(
                        out=fv[:, :, 0:24], in0=fv[:, :, 0:24], in1=fv[:, :, 24:48], op=AL.min)
                    nc.vector.tensor_tensor(
                        out=fv[:, :, 0:12], in0=fv[:, :, 0:12], in1=fv[:, :, 12:24], op=AL.min)
                    nc.vector.tensor_tensor(
                        out=fv[:, :, 0:6], in0=fv[:, :, 0:6], in1=fv[:, :, 6:12], op=AL.min)
                    nc.vector.tensor_tensor(
                        out=fv[:, :, 0:3], in0=fv[:, :, 0:3], in1=fv[:, :, 3:6], op=AL.min)
                    # final 3 -> 1 via one contiguous TR (strided 1-elem TTs are slow)
                    nc.vector.tensor_reduce(
                        out=bp_ch[:, kc - BATCH + 1:kc + 1, :], in_=fv[:, :, 0:3],
                        axis=mybir.AxisListType.X, op=AL.min,
                    )
            nc.sync.dma_start(bp_dram[:, t0:t0 + CK, :], bp_ch[:])
            for tc4 in range(4):
                if t0 <= (tc4 + 1) * TC - 1 < t0 + CK:
                    for q in range(4):
                        nc.sync.dma_start(
                            atile[tc4 * NBIN:(tc4 + 1) * NBIN, :, q, :],
                            ah_v4[:, q, tc4 * TC:(tc4 + 1) * TC, :],
                        )

        # ---- AT pass: AT[bin, t] = enc(first-argmax_c alpha[bin, t, :]) ----
        # atile partitions = (tc, b): lane tc*32+b covers t in [tc*TC, (tc+1)*TC)
        if True:
            atmax = ap.tile([128, TC], F32, tag="atmax")
            nc.vector.tensor_reduce(
                out=atmax[:], in_=atile[:], axis=mybir.AxisListType.XY, op=AL.max,
            )
            atmask = ap.tile([128, TC, D], BF16, tag="atmask")
            nc.vector.tensor_tensor(
                out=atmask[:], in0=atile[:].rearrange("p t q c -> p t (q c)"),
                in1=atmax[:].unsqueeze(2).broadcast_to([128, TC, D]), op=AL.is_ge,
            )
            atf = ap.tile([128, TC, D], BF16, tag="atf")
            nc.vector.tensor_tensor(
                out=atf[:], in0=atmask[:],
                in1=at_enc[:].unsqueeze(1).broadcast_to([128, TC, D]),
                op=AL.mult,
            )
            at_all = ap.tile([128, TC], F32, tag="at_all")
            nc.vector.tensor_reduce(
                out=at_all[:], in_=atf[:], axis=mybir.AxisListType.X, op=AL.min,
            )
            # at_dram[bin, 1 + t] = AT[bin, t]
            for tc4 in range(4):
                nc.sync.dma_start(
                    at_dram[:, 1 + tc4 * TC:1 + (tc4 + 1) * TC],
                    at_all[tc4 * NBIN:(tc4 + 1) * NBIN, :],
                )
            # AT[T-1] straight from SBUF (avoids DRAM roundtrip wait)
            at_last0 = pp.tile([NBIN, 1], F32, tag="at_last0")
            nc.sync.dma_start(at_last0[:], at_all[3 * NBIN:4 * NBIN, TC - 1:TC])
        atpool_ctx.__exit__(None, None, None)

        # ---- backward chase ----
        h = pp.tile([NBIN, D], F32, tag="h")
        tagsq = pp.tile([NBIN, T], F32, tag="tagsq")
        junk = pp.tile([NBIN, D], F32, tag="junk")
        nc.vector.tensor_copy(out=tagsq[:, T - 1:T], in_=at_last0[:])
        nc.vector.tensor_tensor(
            out=h[:], in0=iota_bw[:],
            in1=at_last0[:, 0:1].broadcast_to([NBIN, D]), op=AL.is_equal,
        )

        bp_v = bp_dram.rearrange("(b q) t c -> b q t c", q=4)
        bwp_ctx = tc.tile_pool(name="bwp", bufs=3)
        bwp = bwp_ctx.__enter__()
        for c0 in range(T - 1, 0, -CKB):
            ckb = min(CKB, c0)      # bp indices c0, c0-1, ..., c0-ckb+1 (>=1)
            lo = c0 - ckb + 1
            bpb = bwp.tile([NBIN, CKB, D], BF16, tag="bpb")
            for q in range(4):
                nc.sync.dma_start(
                    bpb[:, 0:ckb, Q * q:Q * (q + 1)], bp_v[:, q, lo:c0 + 1, :]
                )
            atp = bwp.tile([NBIN, CKB], F32, tag="atp")
            nc.sync.dma_start(atp[:, 0:ckb], at_dram[:, lo:c0 + 1])
            # merge: bpb2 = bpb * nbm + AT[t-1] * bm   (boundary const-maps)
            bpb2 = bwp.tile([NBIN, CKB, D], F32, tag="bpb2")
            nc.vector.tensor_tensor(
                out=bpb2[:, 0:ckb, :], in0=bpb[:, 0:ckb, :],
                in1=nbm[:, lo:c0 + 1].unsqueeze(2).broadcast_to([NBIN, ckb, D]),
                op=AL.mult,
            )
            atpm = bwp.tile([NBIN, CKB], F32, tag="atpm")
            nc.vector.tensor_tensor(
                out=atpm[:, 0:ckb], in0=atp[:, 0:ckb],
                in1=bm[:, lo:c0 + 1], op=AL.mult,
            )
            nc.vector.tensor_tensor(
                out=bpb2[:, 0:ckb, :], in0=bpb2[:, 0:ckb, :],
                in1=atpm[:, 0:ckb].unsqueeze(2).broadcast_to([NBIN, ckb, D]), op=AL.add,
            )
            for t in range(c0, lo - 1, -1):
                kk = t - lo
                nc.vector.scalar_tensor_tensor(
                    out=junk[:], in0=bpb2[:, kk, :], scalar=1.0, in1=h[:],
                    op0=AL.mult, op1=AL.mult, accum_out=tagsq[:, t - 1:t],
                )
                if t > 1:
                    nc.vector.tensor_tensor(
                        out=h[:], in0=iota_bw[:],
                        in1=tagsq[:, t - 1:t].broadcast_to([NBIN, D]),
                        op=AL.is_equal,
                    )

        bwp_ctx.__exit__(None, None, None)

        # ---- decode (+64) + cast + store ----
        tags_f = pp.tile([NBIN, T], F32, tag="tags_f")
        nc.vector.tensor_scalar(
            out=tags_f[:], in0=tagsq[:], scalar1=64.0, scalar2=None, op0=AL.add,
        )
        tags_i = pp.tile([NBIN, T], I32, tag="tags_i")
        nc.vector.tensor_copy(out=tags_i[:], in_=tags_f[:])
        nc.sync.dma_start(tags_out, tags_i[:])


# ---------------------------------------------------------------------------
# self-contained harness
# ---------------------------------------------------------------------------
import concourse.bacc as bacc
from concourse.bass_utils import run_bass_kernel_spmd

_NC_CACHE: dict[int, object] = {}


def _input_specs(t_pack):
    return {
        "emit": ([128, t_pack, Q], F32),
        "nr_il": ([128, t_pack], F32),
        "bm": ([NBIN, t_pack], F32),
        "nbm": ([NBIN, t_pack], F32),
        "trans_q": ([128, Q * D], F32),
        "iota_q": ([128, Q * D], BF16),
        "at_enc": ([128, D], F32),
        "iota_bw": ([NBIN, D], F32),
    }


def _build_nc(t_pack):
    if t_pack in _NC_CACHE:
        return _NC_CACHE[t_pack]
    nc = bacc.Bacc(
        "TRN2",
        target_bir_lowering=False,
        debug=False,
        enable_asserts=True,
        num_devices=N_CORES,
    )
    ins = {
        name: nc.dram_tensor(name, shape, dt, kind="ExternalInput").ap()
        for name, (shape, dt) in _input_specs(t_pack).items()
    }
    outs = {"tags": nc.dram_tensor("tags", [NBIN, t_pack], I32, kind="ExternalOutput").ap()}
    with TileContext(nc) as tc:
        crf_kernel(tc, outs, ins, T=t_pack)
    nc.compile()
    _NC_CACHE[t_pack] = nc
    return nc


def _prepare(logits, sent_lengths, crf_params):
    logits = np.asarray(logits, dtype=np.float32)
    lengths = np.asarray(sent_lengths).astype(np.int64)
    t_pack, bins = pack_bins(lengths)
    consts = make_consts(crf_params, t_pack)
    in_maps = []
    for core in range(N_CORES):
        bins_core = bins[core * NBIN:(core + 1) * NBIN]
        in_maps.append(make_core_inputs(logits, bins_core, consts, t_pack))
    return t_pack, bins, in_maps


def _unpack(results, bins, lengths, t_pack):
    out = np.zeros((B, T_IN), dtype=np.int32)
    for core in range(N_CORES):
        tags = results[core]["tags"]            # [NBIN, t_pack] i32
        for bl, seqs in enumerate(bins[core * NBIN:(core + 1) * NBIN]):
            for (idx, s, L) in seqs:
                out[idx, 0:L] = tags[bl, s:s + L]
    return out


def kernel(logits, sent_lengths, crf_params):
    lengths = np.asarray(sent_lengths).astype(np.int64)
    t_pack, bins, in_maps = _prepare(logits, sent_lengths, crf_params)
    nc = _build_nc(t_pack)
    br = run_bass_kernel_spmd(nc, in_maps, core_ids=list(range(N_CORES)))
    return _unpack(br.results, bins, lengths, t_pack)
